# revision 8
# baseline (speedup 1.0000x reference)
"""Trainium2 Bass kernel for nn_ContrastiveLoss (B=32, C*H*W=262144).

Strategy: shard the flattened feature dim N=262144 across 8 cores (32768
elems/sample/core). Each core's slice is staged host-side into a k-major
fp8e4m3 layout (partition = k-lane within 128-chunk, free = chunk*32 +
sample); the three tensors are stored block-wise per group
([x1_W | x2_W | m_W]). Each group issues TWO DMAs: [x1|x2] (gates the
sigmoid) from the sync DGE, and [m] (only needed by the mul, ~2us later)
from the gpsimd software-DGE. DMA transfers get starved ~2.5x once the
engines start hammering SBUF, so the sigmoid chain is paced by cumulative
xy-arrival; splitting the mask out cuts that gating traffic by 1/3.

Per core the kernel computes PSUM-accumulated gram matrices:
  psum_a [128,256] = s1.T@[s1|s2]   (sq1 diag + cross)
  psum_b [128,128] = s2.T@s2        (sq2 diag)
  psum_c [128,128] = z.T@z,  z=(s1-s2)*m  (pos-MSE diag)
with s* = sigmoid(x*) from ONE activation instr per group. All element-wise
ops use CONTIGUOUS 2D access patterns (strided APs run ~2.3x slower on
DVE/Pool), with the k-tile interleaving pushed into the matmul APs (free
for the PE). sub+mul for a few whole big groups run on Pool (GPSIMD pays
~640ns Q7 launch per instr, so no column-splitting), the rest on DVE. fp8
DoubleRow matmuls keep the PE at ~12-16us. Two PSUM sets let the first
set's copies + output DMA overlap the tail groups' compute.

The [128,1024] fp16 partials are DMA'd out; the host folds the 4-chunk
block structure, sums over cores and sets, and applies the tiny exp/log
epilogue.
"""

import numpy as np

TAU = 0.1
B = 32
N = 262144
NCORES = 8
NC_CHUNK = N // NCORES  # elems per sample per core
COLS = NC_CHUNK // 128 * B  # 8192 staged cols per core per tensor
# Tapered group sizes (multiples of 256): small first group so the pipeline
# starts fast, ramp down so the tail chain is short.
GROUPS = [256, 512, 768, 1024, 1024, 1024, 1024, 1024, 768, 512, 256]
# Groups whose sub+mul run entirely on Pool (GPSIMD).
GP_GROUPS = {2, 4, 6, 8, 10}
# groups accumulated into the second psum set, so the first set's copies and
# output DMA overlap the remaining compute instead of trailing it.
SET2_START = 9

_CACHE = {}
LAST_RESULTS = None  # BassKernelResults of the most recent run (for profiling)


def _build_nc():
    import concourse.bacc as bacc
    import concourse.tile as tile
    from concourse import mybir

    assert sum(GROUPS) == COLS
    assert all(g % 256 == 0 for g in GROUPS)
    f32 = mybir.dt.float32
    fp16 = mybir.dt.float16
    fp8 = mybir.dt.float8e4
    sigmoid = mybir.ActivationFunctionType.Sigmoid
    DR = mybir.MatmulPerfMode.DoubleRow

    offs = [0]
    for W in GROUPS:
        offs.append(offs[-1] + W)

    nc = bacc.Bacc(
        "TRN2", target_bir_lowering=False, debug=False, num_devices=NCORES
    )
    xind = nc.dram_tensor("xin", [128, 3 * COLS], fp8, kind="ExternalInput")
    outd = nc.dram_tensor("partials", [128, 1024], fp16, kind="ExternalOutput")

    with tile.TileContext(nc) as tc:
        with (
            tc.tile_pool(name="data", bufs=1) as data,
            tc.tile_pool(name="acc", bufs=1, space="PSUM") as acc,
        ):
            ings, mts = [], []
            for gi, W in enumerate(GROUPS):
                ings.append(
                    data.tile([128, 2 * W], fp8, tag=f"in{gi}", name=f"in{gi}")
                )
                mts.append(data.tile([128, W], fp8, tag=f"mk{gi}", name=f"mk{gi}"))

            # xy DMAs in group order from sync: these pace the sigmoid chain.
            for gi, W in enumerate(GROUPS):
                nc.sync.dma_start(
                    ings[gi][:], xind[:, 3 * offs[gi] : 3 * offs[gi] + 2 * W]
                )

            # mask DMAs via gpsimd software-DGE, interleaved with its
            # sub/mul groups below (descgen ~650ns each, GP is idle early).
            def mask_dma(gi):
                W = GROUPS[gi]
                nc.gpsimd.dma_start(
                    mts[gi][:],
                    xind[:, 3 * offs[gi] + 2 * W : 3 * offs[gi] + 3 * W],
                )

            for gi in range(0, 5):
                mask_dma(gi)

            psums = []
            for s in range(2):
                # separate full banks: PSUM start_tensor_calc zeroes a whole
                # bank region, so accumulators must not share a bank.
                pat = acc.tile([128, 512], f32, tag=f"pa{s}", name=f"pa{s}")
                pbt = acc.tile([128, 512], f32, tag=f"pb{s}", name=f"pb{s}")
                pct = acc.tile([128, 512], f32, tag=f"pc{s}", name=f"pc{s}")
                psums.append((pat[:, 0:256], pbt[:, 0:128], pct[:, 0:128]))
            out_t = data.tile([128, 1024], fp16, tag="out")

            set_pairs = [0, 0]
            for gi, W in enumerate(GROUPS):
                set_pairs[0 if gi < SET2_START else 1] += W // 256

            def flush(s):
                """copy psum set s to SBUF (DVE + ACT in parallel) + DMA."""
                a, b, c = psums[s]
                base = 512 * s
                nc.vector.tensor_copy(out_t[:, base : base + 256], a)
                nc.scalar.copy(out_t[:, base + 256 : base + 384], b)
                nc.scalar.copy(out_t[:, base + 384 : base + 512], c)
                nc.sync.dma_start(
                    outd[:, base : base + 512], out_t[:, base : base + 512]
                )

            pj = 0
            for gi, W in enumerate(GROUPS):
                nj = W // 256
                si = 0 if gi < SET2_START else 1
                if gi == SET2_START:
                    pj = 0
                psum_a, psum_b, psum_c = psums[si]

                # one sigmoid instr per group, plain contiguous in/out:
                # sg = [s1_W | s2_W] blocks, each k-major (i f).
                sg = data.tile([128, 2 * W], fp8, tag=f"s{gi}", name=f"s{gi}")
                nc.scalar.activation(sg[:], ings[gi][:], sigmoid)

                s1 = sg[:, 0:W]
                s2 = sg[:, W : 2 * W]
                dg = data.tile([128, W], fp8, tag=f"d{gi}", name=f"d{gi}")
                zg = data.tile([128, W], fp8, tag=f"z{gi}", name=f"z{gi}")
                # whole-group engine ownership, contiguous full-width ops
                eng = nc.gpsimd if gi in GP_GROUPS else nc.vector
                eng.tensor_sub(dg[:], s1, s2)
                eng.tensor_mul(zg[:], dg[:], mts[gi][:])
                # stagger the remaining mask descgens through GP's program
                if gi == 3:
                    for g2 in range(5, 8):
                        mask_dma(g2)
                elif gi == 5:
                    for g2 in range(8, 11):
                        mask_dma(g2)

                # DR matmul views over the block layout: pair j covers
                # k-tiles 2j, 2j+1. h = s1/s2 block, i = tile-in-pair.
                sgv = sg[:].rearrange("p (h j i f) -> p j i h f", h=2, i=2, f=128)
                for j in range(nj):
                    first = pj == 0
                    last = pj == set_pairs[si] - 1
                    # w_a: s1 tiles [p, i, f]; rhs_a 4D [p, i(plane), h, f]
                    w_a = sgv[:, j, :, 0]
                    w_b = sgv[:, j, :, 1]
                    rhs_a = sgv[:, j]
                    nc.tensor.matmul(
                        psum_a, w_a, rhs_a, start=first, stop=last, perf_mode=DR
                    )
                    nc.tensor.matmul(
                        psum_b, w_b, w_b, start=first, stop=last, perf_mode=DR
                    )
                    w_c = zg[:, j * 256 : (j + 1) * 256].rearrange(
                        "p (i f) -> p i f", i=2
                    )
                    nc.tensor.matmul(
                        psum_c, w_c, w_c, start=first, stop=last, perf_mode=DR
                    )
                    pj += 1
            flush(0)
            flush(1)

    nc.compile()
    return nc


def _get_nc():
    if "nc" not in _CACHE:
        _CACHE["nc"] = _build_nc()
    return _CACHE["nc"]


def _kmajor(full_flat: np.ndarray, c: int) -> np.ndarray:
    """[B, N] float32 -> per-core k-major layout [128, COLS] float32."""
    chunk = full_flat[:, c * NC_CHUNK : (c + 1) * NC_CHUNK]
    return (
        chunk.reshape(B, NC_CHUNK // 128, 128).transpose(2, 1, 0).reshape(128, COLS)
    )


def _stage_core(f1, f2, fm, c, fp8dt) -> np.ndarray:
    """Store the three k-major tensors block-wise per group: [x1_W|x2_W|m_W]."""
    t1 = _kmajor(f1, c)
    t2 = _kmajor(f2, c)
    tm = _kmajor(fm, c)
    out = np.empty((128, 3 * COLS), dtype=fp8dt)
    o = 0
    for W in GROUPS:
        out[:, 3 * o : 3 * o + W] = t1[:, o : o + W]
        out[:, 3 * o + W : 3 * o + 2 * W] = t2[:, o : o + W]
        out[:, 3 * o + 2 * W : 3 * o + 3 * W] = tm[:, o : o + W]
        o += W
    return out


def _host_combine(partials_list):
    sq1 = np.zeros(B, np.float64)
    sq2 = np.zeros(B, np.float64)
    pos = np.zeros(B, np.float64)
    cross = np.zeros((B, B), np.float64)
    for Pfull in partials_list:
        for s in range(2):
            P = Pfull[:, 512 * s : 512 * (s + 1)]
            g1 = P[:, 0:128]
            cr = P[:, 128:256]
            g2 = P[:, 256:384]
            gy = P[:, 384:512]
            for a in range(4):
                blk = slice(a * 32, (a + 1) * 32)
                cross += cr[blk, blk]
                sq1 += np.diagonal(g1[blk, blk])
                sq2 += np.diagonal(g2[blk, blk])
                pos += np.diagonal(gy[blk, blk])
    sim_pos = np.exp(-(pos / N) / TAU)
    d = (sq1[:, None] + sq2[None, :] - 2.0 * cross) / N
    sim = np.exp(-d / TAU)
    sim_neg = sim.sum(axis=1) - np.diagonal(sim)
    loss = -np.log(sim_pos / (sim_pos + sim_neg))
    return np.asarray(loss.mean(), dtype=np.float32)


def kernel(input1: np.ndarray, input2: np.ndarray, mask: np.ndarray) -> np.ndarray:
    global LAST_RESULTS
    import ml_dtypes

    from concourse.bass_utils import run_bass_kernel_spmd

    f1 = np.asarray(input1, dtype=np.float32).reshape(B, N)
    f2 = np.asarray(input2, dtype=np.float32).reshape(B, N)
    fm = np.asarray(mask, dtype=np.float32).reshape(B, N)

    fp8dt = ml_dtypes.float8_e4m3
    in_maps = [
        {"xin": _stage_core(f1, f2, fm, c, fp8dt)} for c in range(NCORES)
    ]
    nc = _get_nc()
    LAST_RESULTS = run_bass_kernel_spmd(nc, in_maps, list(range(NCORES)))
    partials = [LAST_RESULTS.results[c]["partials"] for c in range(NCORES)]
    return _host_combine(partials)
